# revision 5
# baseline (speedup 1.0000x reference)
"""LFR (low-frame-rate) stacking kernel for Trainium2.

Problem: x (64, 4096, 80) f32, input_lens (64,) int32 (lens[0] pinned to 4096).
M=7 window, N=6 stride, LEFT=3 head pad. Output: (64, 683, 560) f32 + new_len (64,) i32.

Per sample with length L (tau = 3 + L):
  out frame f, slot m  <-  g(6f+m) where
    g(t) = x[b, 0]      for t < 3
           x[b, t-3]    for 3 <= t < tau
           x[b, 4095]   for t >= tau
As flat rows r in [0, 4781):  t(r) = r - r//7 (non-decreasing), so the
"broadcast tail" region is exactly a suffix of rows [r0, 4781).

Strategy (pure data-parallel, 8 samples per core, one SPMD program):
  1. static head DMAs (rows 0..6)
  2. static bulk unfold DMA (frames 1..682, overlapping strided read)
     -> its prefix is correct; the ragged suffix is garbage
  3. indirect-scatter overwrite of rows [r0, 4781) with x[b, 4095]:
     - coarse: 64-row-ALIGNED chunks, offsets = chunk indices into a
       chunk view of out; in_ = physically replicated (128, 5120) tile
       built by the vector engine (log-doubling) - off the DMA path
     - fine: single rows for the unaligned head/tail of the region
     unused slots disabled via OOB offsets (oob_is_err=False).
All raggedness lives in tiny int32 offset tables -> the Bass program is
input-independent and compiled once per process.
"""
import sys

if '/opt/trn_rl_repo' not in sys.path:
    sys.path.insert(0, '/opt/trn_rl_repo')

from contextlib import ExitStack

import numpy as np

import concourse.bass as bass
import concourse.mybir as mybir
from concourse.ap import AP
from concourse.bass import IndirectOffsetOnAxis
from concourse.bass_utils import run_bass_kernel_spmd

# geometry constants
B, T, D = 64, 4096, 80
M, N, LEFT = 7, 6, 3
NF = 683                # output frames per sample
NR = NF * M             # 4781 flat output rows (80 f32 each) per sample
BPC = 8                 # samples per core
NCORES = 8
XS = T * D              # elements per sample of x
OS = NF * M * D         # elements per sample of out
NROWS = BPC * NR        # scatter-addressable rows per core
CH = 64                 # coarse chunk rows (aligned grid over core rows)
CHE = CH * D            # elements per chunk (5120)
NCHUNK = NROWS // CH    # 597 full chunks
SENTINEL = 10_000_000   # OOB index -> slot skipped
NSLOT_C = 96            # coarse slots (max needed ~76)
NSLOT_F = 128           # fine slots (max needed ~126)

_CACHED = {}


def _build_program():
    nc = bass.Bass()
    f32, i32 = mybir.dt.float32, mybir.dt.int32
    x = nc.dram_tensor("x", [BPC, T, D], f32, kind="ExternalInput")
    tco = nc.dram_tensor("tco", [BPC, NSLOT_C], i32, kind="ExternalInput")
    tfo = nc.dram_tensor("tfo", [BPC, NSLOT_F], i32, kind="ExternalInput")
    out = nc.dram_tensor("out", [BPC, NF, M * D], f32, kind="ExternalOutput")

    with (
        # per-sample column slices -> no reuse races between samples.
        # big[p, b*CHE : (b+1)*CHE] = x[b,4095,:] replicated CH times.
        nc.sbuf_tensor([128, BPC * CHE], f32) as big,
        nc.sbuf_tensor([NSLOT_C, BPC], i32) as ocs,
        nc.sbuf_tensor([NSLOT_F, BPC], i32) as ofs,
        nc.semaphore("prep_sem") as prep_sem,
        nc.semaphore("dve_sem") as dve_sem,
        nc.semaphore("tail_sem") as tail_sem,
        ExitStack() as _es,
        nc.Block() as block,
    ):
        bulk_sems = [_es.enter_context(nc.semaphore(f"bs{b}")) for b in range(BPC)]
        out_rows = AP(out, 0, [[D, NROWS], [1, D]])          # (38248, 80)
        out_chunks = AP(out, 0, [[CHE, NCHUNK], [1, CHE]])   # (597, 5120)

        def emit_static(eng, b, sem):
            xb = b * XS
            ob = b * OS
            # head rows 0-2 <- x[b,0] replicated
            eng.dma_start(
                out=AP(out, ob, [[1, LEFT * D]]),
                in_=AP(x, xb, [[0, LEFT], [1, D]]),
            ).then_inc(sem, 16)
            # head rows 3-6 <- x[b, 0:4]
            eng.dma_start(
                out=AP(out, ob + LEFT * D, [[1, (M - LEFT) * D]]),
                in_=AP(x, xb, [[1, (M - LEFT) * D]]),
            ).then_inc(sem, 16)
            # bulk: frames 1..682, frame f <- x rows 6f-3 .. 6f+3
            eng.dma_start(
                out=AP(out, ob + M * D, [[1, (NF - 1) * M * D]]),
                in_=AP(x, xb + (N - LEFT) * D, [[N * D, NF - 1], [1, M * D]]),
            ).then_inc(sem, 16)

        @block.sync
        def _(se):
            for b in range(0, 4):
                emit_static(se, b, bulk_sems[b])

        @block.scalar
        def _(se):
            for b in range(4, 8):
                emit_static(se, b, bulk_sems[b])

        @block.vector
        def _(ve):
            ve.wait_ge(prep_sem, 48 * BPC)
            for b in range(BPC):
                base = b * CHE
                w = D
                while w < CHE:                       # 80 -> 5120: 6 doublings
                    ve.tensor_copy(
                        big[:, base + w: base + 2 * w], big[:, base: base + w]
                    ).then_inc(dve_sem, 1)
                    w *= 2

        @block.gpsimd
        def _(ge):
            # all preps up front: bcast seed rows + offset tables
            for b in range(BPC):
                ge.dma_start(
                    out=big[:, b * CHE: b * CHE + D],
                    in_=AP(x, b * XS + (T - 1) * D, [[0, 128], [1, D]]),
                ).then_inc(prep_sem, 16)
                ge.dma_start(
                    out=ocs[:, b:b + 1],
                    in_=AP(tco, b * NSLOT_C, [[1, NSLOT_C], [1, 1]]),
                ).then_inc(prep_sem, 16)
                ge.dma_start(
                    out=ofs[:, b:b + 1],
                    in_=AP(tfo, b * NSLOT_F, [[1, NSLOT_F], [1, 1]]),
                ).then_inc(prep_sem, 16)
            ge.wait_ge(dve_sem, 6 * BPC)   # replicated tiles ready
            for b in range(BPC):
                # wait for THIS sample's static writes (3 DMAs, own sem)
                ge.wait_ge(bulk_sems[b], 48)
                # coarse: aligned 64-row chunks
                ge.indirect_dma_start(
                    out=out_chunks,
                    out_offset=IndirectOffsetOnAxis(ap=ocs[:, b:b + 1], axis=0),
                    in_=big[0:NSLOT_C, b * CHE:(b + 1) * CHE],
                    in_offset=None,
                    bounds_check=NCHUNK - 1,
                    oob_is_err=False,
                ).then_inc(tail_sem, 16)
                # fine: single rows at the unaligned edges
                ge.indirect_dma_start(
                    out=out_rows,
                    out_offset=IndirectOffsetOnAxis(ap=ofs[:, b:b + 1], axis=0),
                    in_=big[0:NSLOT_F, b * CHE: b * CHE + D],
                    in_offset=None,
                    bounds_check=NROWS - 1,
                    oob_is_err=False,
                ).then_inc(tail_sem, 16)
            ge.wait_ge(tail_sem, 32 * BPC)

    return nc


def _host_tables(lens: np.ndarray):
    """Per-sample scatter offset tables.

    Coarse table: indices of 64-row-aligned chunks fully inside the tail.
    Fine table: global row indices for the unaligned head/tail remainder.
    """
    lens = lens.astype(np.int64)
    tau = LEFT + lens                                     # first bcast t
    r = np.arange(NR)
    t_of_r = r - r // M                                   # t(r), non-decreasing
    ge = t_of_r[None, :] >= tau[:, None]                  # (B, NR)
    any_ge = ge.any(axis=1)
    r0 = np.where(any_ge, ge.argmax(axis=1), NR)
    r0 = np.minimum(r0, NR - 1)                           # row 4780 always = x[4095]

    tco = np.full((B, NSLOT_C), SENTINEL, dtype=np.int32)
    tfo = np.full((B, NSLOT_F), SENTINEL, dtype=np.int32)
    for g in range(B):
        b = g % BPC
        g0 = b * NR + int(r0[g])       # first tail row (global in core)
        g1 = (b + 1) * NR              # one past last tail row
        c_lo = -(-g0 // CH)            # ceil
        c_hi = g1 // CH
        fine = []
        if c_lo < c_hi:
            nc_ = c_hi - c_lo
            tco[g, :nc_] = c_lo + np.arange(nc_)
            fine.extend(range(g0, c_lo * CH))
            fine.extend(range(c_hi * CH, g1))
        else:
            fine.extend(range(g0, g1))
        assert len(fine) <= NSLOT_F
        tfo[g, :len(fine)] = fine
    return tco, tfo


def _new_len(lens: np.ndarray) -> np.ndarray:
    # mirror the reference float arithmetic (exact in f32); jnp's f32 // int
    # promotes to int32, so match that output dtype
    lens_f = lens.astype(np.float32)
    n_lfr = np.ceil(lens_f / N)
    prepad = lens_f + LEFT
    delta = prepad - N * (n_lfr - np.float32(1.0))
    right_pad = np.where(M >= delta, M - delta, np.float32(0.0))
    t_all = LEFT + lens_f + right_pad
    return np.floor(t_all / N).astype(np.int32)


def kernel(x, input_lens):
    x = np.ascontiguousarray(np.asarray(x, dtype=np.float32))
    lens = np.asarray(input_lens, dtype=np.int32)
    assert x.shape == (B, T, D) and lens.shape == (B,)

    if "nc" not in _CACHED:
        _CACHED["nc"] = _build_program()
    nc = _CACHED["nc"]

    tco, tfo = _host_tables(lens)
    in_maps = []
    for c in range(NCORES):
        s = slice(c * BPC, (c + 1) * BPC)
        in_maps.append({"x": x[s], "tco": tco[s], "tfo": tfo[s]})

    res = run_bass_kernel_spmd(nc, in_maps, list(range(NCORES)))
    out = np.concatenate([res.results[c]["out"] for c in range(NCORES)], axis=0)
    return out, _new_len(lens)
